# revision 1
# baseline (speedup 1.0000x reference)
"""Dynamic filter layer on 8 trn2 NeuronCores.

out[b,i,j,c] = sum_{di,dj} x[b,i+di,j+dj,c] * flow[b,i,j,di*K+dj]

B=8, H=W=256, C=64, K=5, Ho=Wo=252. Sharding: data-parallel over batch,
one sample per core (SPMD, no collectives).

Per-core algorithm (v3): per tap, on a [rows, 64 cols, 64 ch] chunk:
    tmp_t = x_win * flow_bcast     (DVE tensor_tensor mult; the flow value
                                    is broadcast along c via a step-0 AP)
    psum += S_di.T @ tmp_t         (TensorE matmul, identity shifted by di:
                                    lhsT = I128[:, di:di+124] — does BOTH the
                                    di row-shift and the 25-tap accumulation
                                    in PSUM, for free on the idle PE)
    out_chunk = psum               (ScalarE copy PSUM->SBUF, then DMA)

The row shift lives in the PE weight slice, so only ONE x tile and 5
cheap row-shifted flow copies are needed (engine APs must start at
partition 0, so shifts cannot be partition offsets). tmp rows with no
valid flow row (top block, k < di) are zeroed so 0*x stays finite.
DVE does only 25 long-FD mults per chunk (~5.4us each); adds cost zero
DVE time. The final 4 output rows (252 = 2*124 + 4) use the same scheme
transposed (partition = output column, dj shift via 5 x copies, di on
the free axis).
"""

import numpy as np

H = 256
W = 256
C = 64
K = 5
HO = H - K + 1  # 252
WO = W - K + 1  # 252
NCORES = 8
JW = 64  # column chunk width
BANK_J = 8  # 8 cols x 64 ch = 512 f32 = one PSUM bank

_nc_cache = {}


def _build(reps=1, n_gp=0):
    """reps>1 wraps the whole body in a HW loop (timing calibration only).
    n_gp>0 moves that many taps' multiplies to GpSimd."""
    global _nc_cache
    key = (reps, n_gp)
    if key in _nc_cache:
        return _nc_cache[key]

    import contextlib

    import concourse.bacc as bacc
    import concourse.bass as bass
    import concourse.tile as tile
    from concourse import mybir
    from concourse.masks import make_identity

    f32 = mybir.dt.float32
    mult = mybir.AluOpType.mult
    add = mybir.AluOpType.add

    nc = bacc.Bacc(None, target_bir_lowering=False)
    x = nc.dram_tensor("x", [H, W, C], f32, kind="ExternalInput")
    flow = nc.dram_tensor("flow", [HO, WO, K * K], f32, kind="ExternalInput")
    out = nc.dram_tensor("out", [HO, WO, C], f32, kind="ExternalOutput")

    fbufs = 1 if n_gp else 2

    with tile.TileContext(nc) as tc:
        with (
            tc.tile_pool(name="cst", bufs=1) as cst,
            tc.tile_pool(name="xp", bufs=2) as xp,
            tc.tile_pool(name="fp", bufs=fbufs) as fp,
            tc.tile_pool(name="td", bufs=4) as td,
            tc.tile_pool(name="tg", bufs=2) as tg,
            tc.tile_pool(name="sp", bufs=2) as sp,
            tc.tile_pool(name="pp", bufs=1, space="PSUM") as pp,
        ):
            ident = cst.tile([128, 128], f32, tag="ident")
            make_identity(nc, ident)

            gp_taps = set(range(K * K - n_gp, K * K))

            with tc.For_i(0, reps, 1) if reps > 1 else contextlib.nullcontext():
                # --- main blocks: out rows [0,124) and [124,248) ---
                for i0 in (0, 124):
                    for j0 in range(0, WO, JW):
                        jw = min(JW, WO - j0)
                        xw = min(jw + K - 1, W - j0)
                        xt = xp.tile([128, JW + K - 1, C], f32, tag="x")
                        nc.sync.dma_start(
                            out=xt[:, :xw, :],
                            in_=x[i0 : i0 + 128, j0 : j0 + xw, :],
                        )
                        # fc5[di][k] = flow[i0 + k - di]; rows k < di of the
                        # top block have no source row -> zeroed.
                        fc5 = []
                        for di in range(K):
                            # only taps di*K..di*K+4 are read from this copy
                            ft = fp.tile([128, JW, K], f32, tag=f"f{di}")
                            lo = i0 - di
                            ts0 = di * K
                            if lo >= 0:
                                nc.sync.dma_start(
                                    out=ft[:, :jw, :],
                                    in_=flow[
                                        lo : lo + 128, j0 : j0 + jw,
                                        ts0 : ts0 + K,
                                    ],
                                )
                            else:
                                nc.gpsimd.memset(ft[: -lo, :jw, :], 0.0)
                                nc.sync.dma_start(
                                    out=ft[-lo:, :jw, :],
                                    in_=flow[
                                        0 : 128 + lo, j0 : j0 + jw,
                                        ts0 : ts0 + K,
                                    ],
                                )
                            fc5.append(ft)

                        ps = pp.tile([124, JW, C], f32, tag="ps")
                        for t in range(K * K):
                            di, dj = divmod(t, K)
                            tmp = td.tile([128, JW, C], f32, tag="tmpd")
                            fb = fc5[di][:, :jw, dj : dj + 1].to_broadcast(
                                [128, jw, C]
                            )
                            nc.vector.tensor_tensor(
                                out=tmp[:, :jw, :],
                                in0=xt[:, dj : dj + jw, :],
                                in1=fb,
                                op=mult,
                            )
                            for jj in range(0, jw, BANK_J):
                                njw = min(BANK_J, jw - jj)
                                nc.tensor.matmul(
                                    ps[:, jj : jj + njw, :],
                                    ident[:, di : di + 124],
                                    tmp[:, jj : jj + njw, :],
                                    start=(t == 0),
                                    stop=(t == K * K - 1),
                                )
                        stage = sp.tile([124, JW, C], f32, tag="stage")
                        nc.scalar.copy(out=stage[:, :jw, :], in_=ps[:, :jw, :])
                        nc.sync.dma_start(
                            out=out[i0 : i0 + 124, j0 : j0 + jw, :],
                            in_=stage[:, :jw, :],
                        )


                # --- strip: out rows [248,252), transposed (partition=j) ---
                for j0, P in ((0, 124), (124, 124), (248, 4)):
                    xs5 = []
                    for dj in range(K):
                        xs = fp.tile([P, 8, C], f32, tag=f"f{dj}")
                        nc.sync.dma_start(
                            out=xs,
                            in_=x[
                                HO - 4 : HO + 4, j0 + dj : j0 + dj + P, :
                            ].rearrange("r j c -> j r c"),
                        )
                        xs5.append(xs)
                    fs = xp.tile([P, 4, K * K], f32, tag="x")
                    nc.sync.dma_start(
                        out=fs,
                        in_=flow[HO - 4 : HO, j0 : j0 + P, :].rearrange(
                            "i j t -> j i t"
                        ),
                    )
                    ps_s = pp.tile([P, 4, C], f32, tag="ps")
                    for t in range(K * K):
                        di, dj = divmod(t, K)
                        tmp = td.tile([P, 4, C], f32, tag="tmpd")
                        fb = fs[:, :, t : t + 1].to_broadcast([P, 4, C])
                        nc.vector.tensor_tensor(
                            out=tmp,
                            in0=xs5[dj][:, di : di + 4, :],
                            in1=fb,
                            op=mult,
                        )
                        nc.tensor.matmul(
                            ps_s[:, :, :],
                            ident[:P, :P],
                            tmp[:, :, :],
                            start=(t == 0),
                            stop=(t == K * K - 1),
                        )
                    stage = sp.tile([P, 4, C], f32, tag="stage")
                    nc.scalar.copy(out=stage, in_=ps_s)
                    nc.sync.dma_start(
                        out=out[HO - 4 : HO, j0 : j0 + P, :].rearrange(
                            "i j c -> j i c"
                        ),
                        in_=stage,
                    )

    nc.finalize()
    _nc_cache[key] = nc
    return nc


def _run(x, flow, trace=False):
    """x: [8,H,W,C] f32, flow: [8,HO,WO,25] f32 -> (out [8,HO,WO,C], results)"""
    from concourse.bass_utils import run_bass_kernel_spmd

    nc = _build()
    in_maps = [
        {
            "x": np.ascontiguousarray(x[b], dtype=np.float32),
            "flow": np.ascontiguousarray(flow[b], dtype=np.float32),
        }
        for b in range(NCORES)
    ]
    res = run_bass_kernel_spmd(
        nc, in_maps, core_ids=list(range(NCORES)), trace=trace
    )
    out = np.stack([r["out"] for r in res.results], axis=0)
    return out, res


def kernel(x, flow, ksize=None, **_unused):
    x = np.asarray(x, dtype=np.float32)
    flow = np.asarray(flow, dtype=np.float32)
    out, _ = _run(x, flow, trace=False)
    return out



# revision 3
# speedup vs baseline: 1.5592x; 1.5592x over previous
"""Dynamic filter layer on 8 trn2 NeuronCores.

out[b,i,j,c] = sum_{di,dj} x[b,i+di,j+dj,c] * flow[b,i,j,di*K+dj]

B=8, H=W=256, C=64, K=5, Ho=Wo=252. Sharding: data-parallel over batch,
one sample per core (SPMD, no collectives).

Per-core algorithm (v4, bf16 channel-major):
  - Host converts all inputs to bf16 and pre-shifts flow by di
    (flow2[di,r,t,j] = flow[r-di,j,5di+t], zero-padded), so no
    partition-shifted loads are needed on chip.
  - SBUF x is channel-major ([partition=row, C, j], transposed from the
    DMA'd row-major copy by the otherwise-idle ScalarE). With the flow
    broadcast along C sitting in the MIDDLE AP dim, every operand's
    last AP dim is packed 2-byte -> the DVE multiply runs in the 2x_1P
    perf mode (0.5 cyc/elem instead of 1.0 for f32).
  - PE accumulates the 25 taps in PSUM via shift-identity matmuls
    (lhsT = I128[:, di:di+124]); bf16 matmul is 1 cycle/row vs 4 for
    f32, so PE stays under the DVE time.
  - ScalarE copies PSUM back to channel-minor f32 (transposed AP) for
    a fully contiguous output DMA.
  - Column chunks are 64 wide; the last chunk starts at 188 and
    recomputes 4 overlapping columns so PSUM bank alignment is uniform.
  - The final 4 output rows (252 = 2*124 + 4) use the same scheme
    transposed (partition = output column, i on the free axis), in two
    126-column blocks, fed from host-transposed strip copies of x/flow.
"""

import contextlib

import ml_dtypes
import numpy as np

BF16 = ml_dtypes.bfloat16

H = 256
W = 256
C = 64
K = 5
HO = H - K + 1  # 252
WO = W - K + 1  # 252
NCORES = 8
JW = 64  # column chunk width
J0S = (0, 64, 128, WO - JW)  # last chunk overlaps 4 cols (alignment)
SP = 126  # strip column block (2 * 126 = 252)

_nc_cache = {}


def _build(reps=1):
    """reps>1 wraps the whole body in a HW loop (timing calibration only)."""
    global _nc_cache
    key = reps
    if key in _nc_cache:
        return _nc_cache[key]

    import concourse.bacc as bacc
    import concourse.tile as tile
    from concourse import mybir
    from concourse.masks import make_identity

    f32 = mybir.dt.float32
    bf = mybir.dt.bfloat16
    mult = mybir.AluOpType.mult

    nc = bacc.Bacc(None, target_bir_lowering=False)
    x = nc.dram_tensor("x", [H, W, C], bf, kind="ExternalInput")
    flow2 = nc.dram_tensor("flow2", [K, H, K, WO], bf, kind="ExternalInput")
    xs = nc.dram_tensor("xs", [W, C, 8], bf, kind="ExternalInput")
    flows = nc.dram_tensor("flows", [WO, K * K, 4], bf, kind="ExternalInput")
    out = nc.dram_tensor("out", [HO, WO, C], f32, kind="ExternalOutput")

    with tile.TileContext(nc) as tc:
        with (
            tc.tile_pool(name="cst", bufs=1) as cst,
            tc.tile_pool(name="xr", bufs=1) as xr,
            tc.tile_pool(name="xcp", bufs=2) as xcp,
            tc.tile_pool(name="fp", bufs=2) as fp,
            tc.tile_pool(name="td", bufs=3) as td,
            tc.tile_pool(name="sp", bufs=2) as sp,
            tc.tile_pool(name="st", bufs=2) as stp,
            tc.tile_pool(name="pp", bufs=1, space="PSUM") as pp,
        ):
            ident = cst.tile([128, 128], bf, tag="ident")
            make_identity(nc, ident)

            with tc.For_i(0, reps, 1) if reps > 1 else contextlib.nullcontext():
                # --- main blocks: out rows [0,124) and [124,248) ---
                for i0 in (0, 124):
                    xraw = xr.tile([128, W, C], bf, tag="xraw")
                    nc.sync.dma_start(out=xraw, in_=x[i0 : i0 + 128, :, :])
                    xc = xcp.tile([128, C, W], bf, tag="xc")
                    nc.scalar.copy(out=xc, in_=xraw.rearrange("p j c -> p c j"))
                    f5 = []
                    for di in range(K):
                        ft = fp.tile([128, K, WO], bf, tag=f"f{di}")
                        nc.sync.dma_start(
                            out=ft, in_=flow2[di, i0 : i0 + 128, :, :]
                        )
                        f5.append(ft)
                    for j0 in J0S:
                        ps = pp.tile([124, C, JW], f32, tag="ps")
                        for t in range(K * K):
                            di, dj = divmod(t, K)
                            tmp = td.tile([128, C, JW], bf, tag="tmp")
                            fb = f5[di][
                                :, dj : dj + 1, j0 : j0 + JW
                            ].to_broadcast([128, C, JW])
                            nc.vector.tensor_tensor(
                                out=tmp,
                                in0=xc[:, :, j0 + dj : j0 + dj + JW],
                                in1=fb,
                                op=mult,
                            )
                            for c8 in range(0, C, 8):
                                nc.tensor.matmul(
                                    ps[:, c8 : c8 + 8, :],
                                    ident[:, di : di + 124],
                                    tmp[:, c8 : c8 + 8, :],
                                    start=(t == 0),
                                    stop=(t == K * K - 1),
                                )
                        stage = sp.tile([124, JW, C], f32, tag="stage")
                        nc.scalar.copy(
                            out=stage, in_=ps.rearrange("p c j -> p j c")
                        )
                        nc.sync.dma_start(
                            out=out[i0 : i0 + 124, j0 : j0 + JW, :], in_=stage
                        )

                # --- strip: out rows [248,252), transposed (partition=j) ---
                for j0 in (0, SP):
                    xs5 = []
                    for dj in range(K):
                        xt = stp.tile([SP, C, 8], bf, tag=f"xs{dj}")
                        nc.sync.dma_start(
                            out=xt, in_=xs[j0 + dj : j0 + dj + SP, :, :]
                        )
                        xs5.append(xt)
                    fs = stp.tile([SP, K * K, 4], bf, tag="fs")
                    nc.sync.dma_start(out=fs, in_=flows[j0 : j0 + SP, :, :])
                    ps_s = pp.tile([SP, C, 4], f32, tag="ps")
                    for t in range(K * K):
                        di, dj = divmod(t, K)
                        tmp = td.tile([SP, C, 4], bf, tag="tmps")
                        fb = fs[:, t : t + 1, :].to_broadcast([SP, C, 4])
                        nc.vector.tensor_tensor(
                            out=tmp,
                            in0=xs5[dj][:, :, di : di + 4],
                            in1=fb,
                            op=mult,
                        )
                        nc.tensor.matmul(
                            ps_s,
                            ident[:SP, :SP],
                            tmp,
                            start=(t == 0),
                            stop=(t == K * K - 1),
                        )
                    stage_s = sp.tile([SP, 4, C], f32, tag="stages")
                    nc.scalar.copy(out=stage_s, in_=ps_s.rearrange("p c i -> p i c"))
                    nc.sync.dma_start(
                        out=out[HO - 4 : HO, j0 : j0 + SP, :].rearrange(
                            "i j c -> j i c"
                        ),
                        in_=stage_s,
                    )

    nc.finalize()
    _nc_cache[key] = nc
    return nc


def _prepare_inmaps(x, flow):
    """Host-side prep: bf16 conversion + layout transforms per sample."""
    x = np.asarray(x, dtype=np.float32)
    flow = np.asarray(flow, dtype=np.float32)
    maps = []
    for b in range(NCORES):
        xb = x[b]
        fb = flow[b]
        x16 = np.ascontiguousarray(xb.astype(BF16))
        f2 = np.zeros((K, H, K, WO), dtype=BF16)
        for di in range(K):
            # flow2[di, r, t, j] = flow[r - di, j, 5*di + t]
            f2[di, di : di + HO, :, :] = (
                fb[:, :, K * di : K * di + K].transpose(0, 2, 1).astype(BF16)
            )
        # strip x: rows 248..255 as [j, c, i]
        xsb = np.ascontiguousarray(
            xb[HO - 4 : HO + 4].transpose(1, 2, 0).astype(BF16)
        )
        # strip flow: rows 248..251 as [j, t, i]
        fsb = np.ascontiguousarray(
            fb[HO - 4 : HO].transpose(1, 2, 0).astype(BF16)
        )
        maps.append({"x": x16, "flow2": f2, "xs": xsb, "flows": fsb})
    return maps


def _run(x, flow, trace=False):
    """x: [8,H,W,C] f32, flow: [8,HO,WO,25] f32 -> (out [8,HO,WO,C], results)"""
    from concourse.bass_utils import run_bass_kernel_spmd

    nc = _build()
    in_maps = _prepare_inmaps(x, flow)
    res = run_bass_kernel_spmd(
        nc, in_maps, core_ids=list(range(NCORES)), trace=trace
    )
    out = np.stack([r["out"] for r in res.results], axis=0)
    return out, res


def kernel(x, flow, ksize=None, **_unused):
    x = np.asarray(x, dtype=np.float32)
    flow = np.asarray(flow, dtype=np.float32)
    out, _ = _run(x, flow, trace=False)
    return out
